# revision 24
# baseline (speedup 1.0000x reference)
"""Cross-attention Trainium2 kernel (Bass/Tile), 8-core SPMD.

Math (per batch b):
  q = Wq @ x[b]; k = Wk @ y[b]; v = Wv @ y[b]        (1x1 convs, channel GEMMs)
  q,k L2-normalized per (head,dim) row over the full spatial axis, q scaled by 10
  sim[h,i,j] = sum_d qh[h,d,i] kh[h,d,j]
  out[h,d,i] = sum_j softmax_j(sim)[i,j] v[h,d,j]
  res = Wout @ out + bias

Sharding: batch (2) x spatial-quarters of the query axis (4) -> 8 cores.
Each core gets the full x[b], y[b] (needed for the norms and for k/v), plus its
1024-column query slice, and produces res[:, islice] ([128, 1024]).

Device-side design (per core):
  - All attention work is done in the transposed domain: sim_T[j, i] tiles
    ([128 j, 512 i] slabs) come straight out of the PE, so no transposes of the
    attention matrix are ever needed.
  - softmax has no max-subtraction: q,k are unit rows scaled by 10, so
    |sim| < ~0.2 (verified empirically; worst-case bound ~320 < exp overflow).
  - exp runs on ScalarE directly from PSUM (fusing PSUM evacuation with the
    transcendental); this engine is the throughput floor of the whole kernel
    (16.7M exps/core at 128/cycle @ 1.2 GHz).
  - attn@v uses a stationary [v_h^T | ones] (33-column) operand: PSUM rows
    0..32 accumulate the numerator, row 32 the softmax denominator.
  - The per-query reciprocal denominator is broadcast across partitions with a
    K=1 ones-matmul, then folded in with one vector multiply.
  - float32r matmuls (full PE rate at N=512) keep everything fp32-precision.
"""

import os

import numpy as np

HEADS = 4
DH = 32
HID = 128
SCALE = 10.0
B, C, H, W = 2, 128, 64, 64
N = H * W  # 4096 spatial positions
NCORES = 8
NI = N // 4  # query columns per core
NJT = N // 128  # 32 key j-tiles
SLAB = 512  # i-extent of one psum slab
RND = 3  # slabs per round (3 banks); 2 rounds in flight + 2-bank attn psum = 8 banks

_CACHE = {}

# Benchmark hook: KREPS>1 emits the kernel body multiple times in one NEFF so
# (T(K) - T(1)) / (K - 1) isolates HW body time from dispatch overhead.
REPS = int(os.environ.get("KREPS", "1"))


def _build_program(reps=None):
    import concourse.bacc as bacc
    import concourse.mybir as mybir
    import concourse.tile as tile
    from concourse.bass import ts

    F32 = mybir.dt.float32
    F32R = mybir.dt.float32r
    BF16 = mybir.dt.bfloat16
    AF = mybir.ActivationFunctionType
    OP = mybir.AluOpType

    if reps is None:
        reps = REPS
    nc = bacc.Bacc("TRN2", target_bir_lowering=False, debug=False,
                   num_devices=NCORES)

    xb = nc.dram_tensor("xb", [C, N], F32R, kind="ExternalInput")
    qx = nc.dram_tensor("qx", [C, NI], F32R, kind="ExternalInput")
    yb = nc.dram_tensor("yb", [C, N], F32R, kind="ExternalInput")
    wq = nc.dram_tensor("wq", [C, HID], F32R, kind="ExternalInput")  # Wq^T
    wk = nc.dram_tensor("wk", [C, HID], F32R, kind="ExternalInput")  # Wk^T
    wv = nc.dram_tensor("wv", [C, HID], F32R, kind="ExternalInput")  # Wv^T
    wo = nc.dram_tensor("wo", [HID, C], F32R, kind="ExternalInput")  # Wout^T
    bo = nc.dram_tensor("bo", [C, 1], F32, kind="ExternalInput")
    out_d = nc.dram_tensor("out", [C, NI], F32, kind="ExternalOutput")

    with tile.TileContext(nc) as tc:
        with (
            tc.tile_pool(name="singles", bufs=1) as singles,
            tc.tile_pool(name="small", bufs=4) as small,
            tc.tile_pool(name="scr", bufs=2) as scr,
            tc.tile_pool(name="pT", bufs=4) as pTp,
            tc.tile_pool(name="psR", bufs=2, space="PSUM") as psR,
            tc.tile_pool(name="psO", bufs=2, space="PSUM") as psO,
        ):
          for _rep in range(reps):
        # ---- input loads, chunked and spread across DMA queues so the
                # first projection matmuls can start before the full loads land
                xb_t = singles.tile([C, N], F32R)
                qx_t = singles.tile([C, NI], F32R)
                yb_t = singles.tile([C, N], F32R)
                wq_t = singles.tile([C, HID], F32R)
                wk_t = singles.tile([C, HID], F32R)
                wv_t = singles.tile([C, HID], F32R)
                wo_t = singles.tile([HID, C], F32R)
                bo_t = singles.tile([C, 1], F32)
                nc.gpsimd.dma_start(wk_t[:], wk[:])
                nc.gpsimd.dma_start(yb_t[:, ts(0, SLAB)], yb[:, ts(0, SLAB)])
                nc.gpsimd.dma_start(xb_t[:, ts(0, SLAB)], xb[:, ts(0, SLAB)])
                nc.gpsimd.dma_start(wv_t[:], wv[:])
                nc.gpsimd.dma_start(wq_t[:], wq[:])
                for n in range(1, 8):
                    ya, yb_e = (nc.sync, nc.scalar) if n % 2 == 0 else (nc.scalar, nc.sync)
                    ya.dma_start(yb_t[:, ts(n, SLAB)], yb[:, ts(n, SLAB)])
                    yb_e.dma_start(xb_t[:, ts(n, SLAB)], xb[:, ts(n, SLAB)])
                nc.gpsimd.dma_start(qx_t[:], qx[:])
                nc.gpsimd.dma_start(wo_t[:], wo[:])
                nc.gpsimd.dma_start(bo_t[:], bo[:])

                # pin the exp_and_friends ACT table (Exp/Square/Copy) once,
                # before any activation, so no mid-kernel table switches occur
                dummy = scr.tile([1, 1], F32, tag="dummy")
                nc.vector.memset(dummy[:], 0.0)
                nc.scalar.activation(dummy[:], dummy[:], AF.Exp)

                # ---- k projection into one [128, N] tile + sum-of-squares
                # (Square on ScalarE doubles as the norm-path psum read);
                # v^T batches interleave on the same y chunks: out[j, hd] =
                # sum_c y[c, j] WvT[c, hd], scattered into [v^T | ones] layout
                kh_t = singles.tile([C, N], F32R)
                ssk_parts = singles.tile([C, 8], F32)
                vTa = singles.tile([128, NJT, HEADS, DH + 1], F32R)
                ones_scr = scr.tile([128, NJT, HEADS, 1], F32, tag="ones")
                nc.vector.memset(ones_scr[:], 1.0)
                nc.vector.tensor_copy(vTa[:, :, :, DH : DH + 1], ones_scr[:])
                for n in range(8):
                    pk = psR.tile([C, SLAB], F32, tag="rnd")
                    nc.tensor.matmul(pk[:], wk_t[:], yb_t[:, ts(n, SLAB)],
                                     start=True, stop=True)
                    nc.vector.tensor_copy(kh_t[:, ts(n, SLAB)], pk[:])
                    sqk_scr = scr.tile([C, SLAB], BF16, tag="sqscr")
                    nc.scalar.activation(sqk_scr[:], pk[:], AF.Square,
                                         accum_out=ssk_parts[:, n : n + 1])
                    pv = psO.tile([128, 4, HID], F32, tag="out")
                    for u in range(4):
                        nc.tensor.matmul(pv[:, u, :],
                                         yb_t[:, ts(4 * n + u, 128)], wv_t[:],
                                         start=True, stop=True)
                    nc.vector.tensor_copy(
                        vTa[:, 4 * n : 4 * n + 4, :, 0:DH],
                        pv[:].rearrange("p a (h d) -> p a h d", h=HEADS),
                    )

                # ---- q: sum-of-squares over the full spatial axis, plus the raw
                # query slice (both norms and the x10 scale are folded into k)
                qh_t = singles.tile([C, NI], F32R)
                ssq_parts = singles.tile([C, 8], F32)
                for n in range(8):
                    pq = psR.tile([C, SLAB], F32, tag="rnd")
                    nc.tensor.matmul(pq[:], wq_t[:], xb_t[:, ts(n, SLAB)],
                                     start=True, stop=True)
                    sqq_scr = scr.tile([C, SLAB], BF16, tag="sqscr")
                    nc.scalar.activation(sqq_scr[:], pq[:], AF.Square,
                                         accum_out=ssq_parts[:, n : n + 1])
                for n in range(NI // SLAB):
                    pq2 = psR.tile([C, SLAB], F32, tag="rnd")
                    nc.tensor.matmul(pq2[:], wq_t[:], qx_t[:, ts(n, SLAB)],
                                     start=True, stop=True)
                    nc.vector.tensor_copy(qh_t[:, ts(n, SLAB)], pq2[:])

                # ---- norm scalars: rqk = 10 / (|q row| * |k row|), folded into k
                ssq = small.tile([C, 1], F32, tag="ss")
                nc.vector.reduce_sum(out=ssq[:], in_=ssq_parts[:],
                                     axis=mybir.AxisListType.X)
                ssk = small.tile([C, 1], F32, tag="ss")
                nc.vector.reduce_sum(out=ssk[:], in_=ssk_parts[:],
                                     axis=mybir.AxisListType.X)
                nqk = small.tile([C, 1], F32, tag="ss")
                nc.vector.tensor_mul(nqk[:], ssq[:], ssk[:])
                # rqk = 1/sqrt(nqk) on DVE: bit-trick seed + 2 Newton steps
                I32 = mybir.dt.int32
                magic = scr.tile([C, 1], I32, tag="magic")
                nc.vector.memset(magic[:], 0x5F3759DF)
                sshalf = small.tile([C, 1], I32, tag="nt")
                nc.vector.tensor_scalar(sshalf[:], nqk[:].bitcast(I32), 1, None,
                                        OP.logical_shift_right)
                y_t = small.tile([C, 1], F32, tag="nt")
                nc.vector.tensor_tensor(out=y_t[:].bitcast(I32), in0=magic[:],
                                        in1=sshalf[:], op=OP.subtract)
                for _ in range(2):
                    yy = small.tile([C, 1], F32, tag="nt")
                    nc.vector.tensor_mul(yy[:], y_t[:], y_t[:])
                    xyy = small.tile([C, 1], F32, tag="nt")
                    nc.vector.tensor_mul(xyy[:], yy[:], nqk[:])
                    cc = small.tile([C, 1], F32, tag="nt")
                    nc.vector.tensor_scalar(cc[:], xyy[:], -0.5, 1.5,
                                            OP.mult, OP.add)
                    yn = small.tile([C, 1], F32, tag="nt")
                    nc.vector.tensor_mul(yn[:], y_t[:], cc[:])
                    y_t = yn
                rqk = y_t
                for n in range(8):
                    nc.vector.tensor_scalar(kh_t[:, ts(n, SLAB)],
                                            kh_t[:, ts(n, SLAB)].bitcast(F32),
                                            rqk[:], SCALE, OP.mult, OP.mult)

                ones32 = singles.tile([1, DH], F32)
                nc.vector.memset(ones32[:], 1.0)
                out_hidden = singles.tile([C, NI], F32R)

                # ---- main loop: per (head, query-half), stream j-tiles through
                # sim_T -> exp -> [v^T | ones] matmul; softmax denominator lands
                # in psum row DH, numerator in rows 0..DH
                def epilogue(h, c, po):
                    # numerator rows / denominator row (emitted one group late so
                    # the PE-queue bcast matmul never waits on the reciprocal)
                    recip = small.tile([1, SLAB], F32, tag="recip")
                    nc.vector.reciprocal(recip[:], po[DH : DH + 1, :])
                    nc.tensor.matmul(po[64 : 64 + DH, :], ones32[:], recip[:],
                                     start=True, stop=True)
                    rb = small.tile([DH, SLAB], F32, tag="rb")
                    nc.vector.tensor_copy(rb[:], po[64 : 64 + DH, :])
                    nc.vector.tensor_mul(out_hidden[ts(h, DH), ts(c, SLAB)],
                                         po[0:DH, :], rb[:])

                pending = None
                pend_av = None  # attn@v of the previous round, emitted one late
                for h in range(HEADS):
                    for c in range(NI // SLAB):
                        po = psO.tile([128, SLAB], F32, tag="out")
                        rounds = [list(range(r, min(r + RND, NJT)))
                                  for r in range(0, NJT, RND)]
                        for ri, rnd in enumerate(rounds):
                            nr = len(rnd)
                            pr = psR.tile([128, RND, SLAB], F32, tag="rnd")
                            pt = pTp.tile([128, RND, SLAB], F32R, tag="pt")
                            for s, jj in enumerate(rnd):
                                nc.tensor.matmul(
                                    pr[:, s, :],
                                    kh_t[ts(h, DH), ts(jj, 128)],
                                    qh_t[ts(h, DH), ts(c, SLAB)],
                                    start=True, stop=True,
                                    tile_position=(h * DH, 0),
                                )
                            nc.scalar.activation(
                                pt[:, 0:nr, :].rearrange("p a b -> p (a b)"),
                                pr[:, 0:nr, :].rearrange("p a b -> p (a b)"),
                                AF.Exp,
                            )
                            if pend_av is not None:
                                av_po, av_h, av_rnd, av_pt = pend_av
                                for s, jj in enumerate(av_rnd):
                                    nc.tensor.matmul(
                                        av_po[0 : DH + 1, :],
                                        vTa[:, jj, av_h, :],
                                        av_pt[:, s, :],
                                        start=(jj == 0), stop=(jj == NJT - 1),
                                    )
                            pend_av = (po, h, rnd, pt)
                            if ri == 1 and pending is not None:
                                epilogue(*pending)
                                pending = None
                        pending = (h, c, po)
                av_po, av_h, av_rnd, av_pt = pend_av
                for s, jj in enumerate(av_rnd):
                    nc.tensor.matmul(
                        av_po[0 : DH + 1, :],
                        vTa[:, jj, av_h, :],
                        av_pt[:, s, :],
                        start=(jj == 0), stop=(jj == NJT - 1),
                    )
                epilogue(*pending)

                # ---- output projection + bias
                res_t = singles.tile([C, NI], F32)
                for c in range(NI // SLAB):
                    pf = psO.tile([128, SLAB], F32, tag="out")
                    nc.tensor.matmul(pf[:], wo_t[:], out_hidden[:, ts(c, SLAB)],
                                     start=True, stop=True)
                    nc.vector.tensor_scalar_add(res_t[:, ts(c, SLAB)], pf[:],
                                                bo_t[:])
                    nc.sync.dma_start(out_d[:, ts(c, SLAB)], res_t[:, ts(c, SLAB)])

    nc.compile()
    return nc


def _get_program(reps=None):
    key = reps if reps is not None else REPS
    if key not in _CACHE:
        _CACHE[key] = _build_program(key)
    return _CACHE[key]


def _prepare_in_maps(x, y, w_qkv, w_out, b_out):
    x = np.ascontiguousarray(np.asarray(x, dtype=np.float32))
    y = np.ascontiguousarray(np.asarray(y, dtype=np.float32))
    w_qkv = np.asarray(w_qkv, dtype=np.float32)
    w_out = np.asarray(w_out, dtype=np.float32)
    b_out = np.asarray(b_out, dtype=np.float32)

    xf = x.reshape(B, C, N)
    yf = y.reshape(B, C, N)
    wq_T = np.ascontiguousarray(w_qkv[0:HID].T)
    wk_T = np.ascontiguousarray(w_qkv[HID : 2 * HID].T)
    wv_T = np.ascontiguousarray(w_qkv[2 * HID :].T)
    wo_T = np.ascontiguousarray(w_out.T)
    bo_v = np.ascontiguousarray(b_out.reshape(C, 1))

    in_maps = []
    for core in range(NCORES):
        b = core // 4
        i0 = (core % 4) * NI
        in_maps.append({
            "xb": np.ascontiguousarray(xf[b]),
            "qx": np.ascontiguousarray(xf[b][:, i0 : i0 + NI]),
            "yb": np.ascontiguousarray(yf[b]),
            "wq": wq_T, "wk": wk_T, "wv": wv_T, "wo": wo_T, "bo": bo_v,
        })
    return in_maps


def _assemble_output(results):
    out = np.empty((B, C, N), dtype=np.float32)
    for core in range(NCORES):
        b = core // 4
        i0 = (core % 4) * NI
        out[b][:, i0 : i0 + NI] = results[core]["out"]
    return out.reshape(B, C, H, W)


def kernel(x, y, w_qkv, w_out, b_out):
    from concourse.bass_utils import run_bass_kernel_spmd

    in_maps = _prepare_in_maps(x, y, w_qkv, w_out, b_out)
    nc = _get_program()
    res = run_bass_kernel_spmd(nc, in_maps, core_ids=list(range(NCORES)))
    return _assemble_output(res.results)

